# revision 6
# baseline (speedup 1.0000x reference)
"""Multi-head attention forward (B=4, N=1024, D=768, H=12, dh=64) on 8 TRN2 cores.

Sharding: (batch, head-group) — core c handles batch b = c//2 and heads
hs..hs+5 where hs = (c%2)*6.  Each core computes its 6 heads' contribution
to out[b] = attn(x[b]) @ W_out_rows(for its heads); host sums the two
partials per batch and adds the bias (the "all-reduce after final linear").

Per-core dataflow (all contraction dims on SBUF partitions):
  qkT  [768,1024] = w_qk^T @ x^T          (d-major q,k — feeds scores)
  v    [1024,390] = x @ w_v (+ ones col)  (n-major v — feeds AV^T)
  S^T  [1024,1024]/head = k_h @ q_h^T     (keys on partitions)
  P^T  = exp(S^T * scale)                 (no max-sub: scores ~ N(0,1))
  oT   [65,1024]/head = [v_h|1]^T @ P^T   (row 64 = softmax denominators)
  attT = oT[0:64] / denom                 (DMA-broadcast denom, DVE mult)
  out  [1024,768] = attT^T @ w_o          (partial; host all-reduce)
"""
import os
import sys

sys.path.insert(0, "/opt/trn_rl_repo")

import numpy as np
import concourse.bass as bass
import concourse.bacc as bacc
import concourse.tile as tile
from concourse import mybir
from concourse.bass_utils import run_bass_kernel_spmd
from contextlib import ExitStack

F32 = mybir.dt.float32
F32R = mybir.dt.float32r

DIM = 768
N = 1024
HEADS_PER_CORE = 6
DH = 64
SCALE = DH ** -0.5
NCORES = 8

# "f32r" = TF32-like matmul mode (4x faster PE, ~1.6e-4 rel err)
# "f32"  = full fp32 matmuls
MODE = os.environ.get("ATTN_MM_DTYPE", "f32")


def build_nc(mode=MODE):
    DT = F32R if mode == "f32r" else F32
    nc = bacc.Bacc("TRN2", target_bir_lowering=False, debug=False)

    xT_d = nc.declare_dram_parameter("xT", [DIM, N], DT, isOutput=False)
    wqk_d = nc.declare_dram_parameter("w_qk", [DIM, 768], DT, isOutput=False)
    wv_d = nc.declare_dram_parameter("w_v", [DIM, 384], DT, isOutput=False)
    wo_d = nc.declare_dram_parameter("w_o", [384, DIM], DT, isOutput=False)
    out_d = nc.declare_dram_parameter("out", [N, DIM], F32, isOutput=True)
    dinv_dram = nc.dram_tensor("dinv_scratch", [12, 512], F32)

    with tile.TileContext(nc) as tc:
        with ExitStack() as ctx:
            persist = ctx.enter_context(tc.tile_pool(name="persist", bufs=1))
            pt_pool = ctx.enter_context(tc.tile_pool(name="pt", bufs=6))
            stats = ctx.enter_context(tc.tile_pool(name="stats", bufs=3))
            outsb = ctx.enter_context(tc.tile_pool(name="outsb", bufs=3))
            ps_mm = ctx.enter_context(tc.tile_pool(name="ps_mm", bufs=4, space="PSUM"))
            ps_acc = ctx.enter_context(tc.tile_pool(name="ps_acc", bufs=4, space="PSUM"))

            xT = persist.tile([128, 6, N], DT)
            wqk = persist.tile([128, 6, 768], DT)
            wv = persist.tile([128, 6, 384], DT)
            wo = persist.tile([128, 3, 768], DT)
            qkT = persist.tile([128, 6, N], DT)
            v_sb = persist.tile([128, 8, 6 * 65], DT)
            attT = persist.tile([128, 3, N], DT)

            # input DMAs, one per k-tile so they spread across DMA queues
            for kt in range(6):
                nc.sync.dma_start(out=xT[:, kt, :], in_=xT_d[kt * 128:(kt + 1) * 128, :])
                nc.sync.dma_start(out=wqk[:, kt, :], in_=wqk_d[kt * 128:(kt + 1) * 128, :])
                nc.sync.dma_start(out=wv[:, kt, :], in_=wv_d[kt * 128:(kt + 1) * 128, :])
            for kt in range(3):
                nc.sync.dma_start(out=wo[:, kt, :], in_=wo_d[kt * 128:(kt + 1) * 128, :])

            # ones columns for the fused softmax denominators
            nc.vector.memset(v_sb[:], 1.0)

            # ---- phase 1: qkT[mt] = (w_qk col-block mt)^T @ xT ----
            for mt in range(6):
                for ch in range(2):
                    ps = ps_mm.tile([128, 512], F32, tag="mm")
                    for kt in range(6):
                        nc.tensor.matmul(
                            ps,
                            wqk[:, kt, mt * 128:(mt + 1) * 128],
                            xT[:, kt, ch * 512:(ch + 1) * 512],
                            start=(kt == 0),
                            stop=(kt == 5),
                        )
                    nc.scalar.copy(qkT[:, mt, ch * 512:(ch + 1) * 512], ps)

            # ---- phase 2: v[i] = x rows-block i @ w_v (strided into v_sb) ----
            for i in range(8):
                ps = ps_mm.tile([128, 384], F32, tag="mm")
                for kt in range(6):
                    nc.tensor.matmul(
                        ps,
                        xT[:, kt, i * 128:(i + 1) * 128],
                        wv[:, kt, :],
                        start=(kt == 0),
                        stop=(kt == 5),
                    )
                dst = v_sb[:, i, :].rearrange("p (h c) -> p h c", h=6)[:, :, 0:DH]
                src = ps.rearrange("p (h c) -> p h c", h=6)
                nc.vector.tensor_copy(dst, src)

            # ---- phase 3: attention, head pairs (2p, 2p+1) row-packed ----
            for p in range(3):
                o_ps = {}  # (head_idx_in_pair, chunk) -> [65, 512] accumulator
                for hp in range(2):
                    for ch in range(2):
                        o_ps[(hp, ch)] = ps_acc.tile(
                            [65, 512], F32, tag="acc", name=f"oacc_p{p}_{hp}_{ch}"
                        )
                for i in range(8):
                    for ch in range(2):
                        pt = {}
                        for hp in range(2):
                            lo, hi = hp * 64, hp * 64 + 64
                            s = ps_mm.tile([128, 512], F32, tag="mm")
                            nc.tensor.matmul(
                                s,
                                qkT[lo:hi, 3 + p, i * 128:(i + 1) * 128],
                                qkT[lo:hi, p, ch * 512:(ch + 1) * 512],
                                start=True,
                                stop=True,
                            )
                            pt[hp] = pt_pool.tile(
                                [128, 512], DT, tag="pt", name=f"pt_{p}_{i}_{ch}_{hp}"
                            )
                            nc.scalar.activation(
                                pt[hp], s, mybir.ActivationFunctionType.Exp,
                                scale=SCALE,
                            )
                        for hp in range(2):
                            h = 2 * p + hp
                            nc.tensor.matmul(
                                o_ps[(hp, ch)],
                                v_sb[:, i, h * 65:h * 65 + 65],
                                pt[hp],
                                start=(i == 0),
                                stop=(i == 7),
                            )
                # normalize: attT rows [hp*64 : hp*64+64] of k-tile p
                for hp in range(2):
                    for ch in range(2):
                        acc = o_ps[(hp, ch)]
                        dinv = stats.tile([65, 512], F32, tag="dinv")
                        nc.vector.reciprocal(dinv[64:65, :], acc[64:65, :])
                        # SBUF sources can't broadcast across partitions;
                        # bounce the denominator row through DRAM.
                        drow = 4 * p + 2 * hp + ch
                        nc.sync.dma_start(
                            out=dinv_dram[drow:drow + 1, :], in_=dinv[64:65, :]
                        )
                        dinv_b = stats.tile([64, 512], F32, tag="dinv_b")
                        nc.sync.dma_start(
                            out=dinv_b,
                            in_=dinv_dram[drow, :].partition_broadcast(64),
                        )
                        if hp == 0:
                            nc.vector.tensor_mul(
                                attT[0:64, p, ch * 512:(ch + 1) * 512],
                                acc[0:64, :],
                                dinv_b,
                            )
                        else:
                            tmp = stats.tile([64, 512], DT, tag="odd_tmp")
                            nc.vector.tensor_mul(tmp, acc[0:64, :], dinv_b)
                            nc.sync.dma_start(
                                out=attT[64:128, p, ch * 512:(ch + 1) * 512],
                                in_=tmp,
                            )

            # ---- phase 4: out = attT^T @ w_o ----
            for i in range(8):
                osb = outsb.tile([128, 768], F32, tag="osb")
                for ch, (c0, cw) in enumerate(((0, 512), (512, 256))):
                    ps = ps_mm.tile([128, 512], F32, tag="mm")
                    for j in range(3):
                        nc.tensor.matmul(
                            ps[:, 0:cw],
                            attT[:, j, i * 128:(i + 1) * 128],
                            wo[:, j, c0:c0 + cw],
                            start=(j == 0),
                            stop=(j == 2),
                        )
                    nc.vector.tensor_copy(osb[:, c0:c0 + cw], ps[:, 0:cw])
                nc.sync.dma_start(out=out_d[i * 128:(i + 1) * 128, :], in_=osb)

    nc.compile()
    return nc


_NC_CACHE = {}


def _get_nc():
    if MODE not in _NC_CACHE:
        _NC_CACHE[MODE] = build_nc(MODE)
    return _NC_CACHE[MODE]


def kernel(x, w_qkv, w_out, b_out):
    x = np.asarray(x, dtype=np.float32)
    w_qkv = np.asarray(w_qkv, dtype=np.float32)
    w_out = np.asarray(w_out, dtype=np.float32)
    b_out = np.asarray(b_out, dtype=np.float32)

    nc = _get_nc()
    in_maps = []
    for c in range(NCORES):
        b = c // 2
        hs = (c % 2) * HEADS_PER_CORE
        q_cols = w_qkv[:, hs * DH:(hs + 6) * DH]
        k_cols = w_qkv[:, 768 + hs * DH:768 + (hs + 6) * DH]
        in_maps.append({
            "xT": np.ascontiguousarray(x[b].T),
            "w_qk": np.ascontiguousarray(np.concatenate([q_cols, k_cols], axis=1)),
            "w_v": np.ascontiguousarray(w_qkv[:, 1536 + hs * DH:1536 + (hs + 6) * DH]),
            "w_o": np.ascontiguousarray(w_out[hs * DH:(hs + 6) * DH, :]),
        })

    res = run_bass_kernel_spmd(nc, in_maps, core_ids=list(range(NCORES))).results

    out = np.empty((4, N, DIM), dtype=np.float32)
    for b in range(4):
        out[b] = res[2 * b]["out"] + res[2 * b + 1]["out"] + b_out
    return out


# revision 8
# speedup vs baseline: 2.4136x; 2.4136x over previous
"""Multi-head attention forward (B=4, N=1024, D=768, H=12, dh=64) on 8 TRN2 cores.

Sharding: (batch, head-group) — core c handles batch b = c//2 and heads
hs..hs+5 where hs = (c%2)*6.  Each core computes its 6 heads' contribution
to out[b] = attn(x[b]) @ W_out_rows(for its heads); host sums the two
partials per batch and adds the bias (the "all-reduce after final linear").

Per-core dataflow (all contraction dims on SBUF partitions):
  qkT  [768,1024] = w_qk^T @ x^T          (d-major q,k — feeds scores)
  v    [1024,390] = x @ w_v (+ ones col)  (n-major v — feeds AV^T)
  S^T  [1024,1024]/head = k_h @ q_h^T     (keys on partitions)
  P^T  = exp(S^T * scale)                 (no max-sub: scores ~ N(0,1))
  oT   [65,1024]/head = [v_h|1]^T @ P^T   (row 64 = softmax denominators)
  attT = oT[0:64] / denom                 (DMA-broadcast denom, DVE mult)
  out  [1024,768] = attT^T @ w_o          (partial; host all-reduce)
"""
import os
import sys

sys.path.insert(0, "/opt/trn_rl_repo")

import numpy as np
import concourse.bass as bass
import concourse.bacc as bacc
import concourse.tile as tile
from concourse import mybir
from concourse.bass_utils import run_bass_kernel_spmd
from contextlib import ExitStack

F32 = mybir.dt.float32
F32R = mybir.dt.float32r

DIM = 768
N = 1024
HEADS_PER_CORE = 6
DH = 64
SCALE = DH ** -0.5
NCORES = 8

# "f32r" = TF32-like matmul mode (4x faster PE, reduced mantissa)
# "f32"  = full fp32 matmuls
MODE = os.environ.get("ATTN_MM_DTYPE", "f32")


def build_nc(mode=MODE):
    DT = F32R if mode == "f32r" else F32
    nc = bacc.Bacc("TRN2", target_bir_lowering=False, debug=False)

    xT_d = nc.declare_dram_parameter("xT", [DIM, N], DT, isOutput=False)
    wqk_d = nc.declare_dram_parameter("w_qk", [DIM, 768], DT, isOutput=False)
    wv_d = nc.declare_dram_parameter("w_v", [DIM, 384], DT, isOutput=False)
    wo_d = nc.declare_dram_parameter("w_o", [384, DIM], DT, isOutput=False)
    ones_d = nc.declare_dram_parameter("ones_col", [128, 48], DT, isOutput=False)
    out_d = nc.declare_dram_parameter("out", [N, DIM], F32, isOutput=True)
    dinv_dram = nc.dram_tensor("dinv_scratch", [12, 512], F32)

    with tile.TileContext(nc) as tc:
        with ExitStack() as ctx:
            persist = ctx.enter_context(tc.tile_pool(name="persist", bufs=1))
            pt_pool = ctx.enter_context(tc.tile_pool(name="pt", bufs=6))
            stats = ctx.enter_context(tc.tile_pool(name="stats", bufs=3))
            outsb = ctx.enter_context(tc.tile_pool(name="outsb", bufs=3))
            ps_mm = ctx.enter_context(tc.tile_pool(name="ps_mm", bufs=4, space="PSUM"))
            ps_acc = ctx.enter_context(tc.tile_pool(name="ps_acc", bufs=4, space="PSUM"))

            xT = persist.tile([128, 6, N], DT)
            wqk = persist.tile([128, 6, 768], DT)
            wv = persist.tile([128, 6, 384], DT)
            wo = persist.tile([128, 3, 768], DT)
            qkT = persist.tile([128, 6, N], DT)
            v_sb = persist.tile([128, 8, 6 * 65], DT)
            attT = persist.tile([128, 3, N], DT)

            # input DMAs, one per k-tile so they spread across DMA queues
            for kt in range(6):
                nc.sync.dma_start(out=xT[:, kt, :], in_=xT_d[kt * 128:(kt + 1) * 128, :])
                nc.sync.dma_start(out=wqk[:, kt, :], in_=wqk_d[kt * 128:(kt + 1) * 128, :])
                nc.sync.dma_start(out=wv[:, kt, :], in_=wv_d[kt * 128:(kt + 1) * 128, :])
            for kt in range(3):
                nc.sync.dma_start(out=wo[:, kt, :], in_=wo_d[kt * 128:(kt + 1) * 128, :])
            # ones columns for the fused softmax denominators:
            # v_sb[:, i, h*65 + 64] = 1.0 for all (i, h)
            v_ones_view = v_sb.rearrange("p i (h c) -> p i h c", h=6)[:, :, :, 64]
            ones_view = ones_d.rearrange("p (i h) -> p i h", i=8)
            nc.sync.dma_start(out=v_ones_view, in_=ones_view)

            # ---- phase 1: qkT[mt] = (w_qk col-block mt)^T @ xT ----
            for mt in range(6):
                for ch in range(2):
                    ps = ps_mm.tile([128, 512], F32, tag="mm")
                    for kt in range(6):
                        nc.tensor.matmul(
                            ps,
                            wqk[:, kt, mt * 128:(mt + 1) * 128],
                            xT[:, kt, ch * 512:(ch + 1) * 512],
                            start=(kt == 0),
                            stop=(kt == 5),
                        )
                    nc.scalar.copy(qkT[:, mt, ch * 512:(ch + 1) * 512], ps)

            # ---- phase 2: v[i] = x rows-block i @ w_v (strided into v_sb) ----
            for i in range(8):
                ps = ps_mm.tile([128, 384], F32, tag="mm")
                for kt in range(6):
                    nc.tensor.matmul(
                        ps,
                        xT[:, kt, i * 128:(i + 1) * 128],
                        wv[:, kt, :],
                        start=(kt == 0),
                        stop=(kt == 5),
                    )
                dst = v_sb[:, i, :].rearrange("p (h c) -> p h c", h=6)[:, :, 0:DH]
                src = ps.rearrange("p (h c) -> p h c", h=6)
                nc.vector.tensor_copy(dst, src)

            # ---- phase 3: attention, head pairs (2p, 2p+1) row-packed ----
            for p in range(3):
                o_ps = {}  # (head_idx_in_pair, chunk) -> [65, 512] accumulator
                for hp in range(2):
                    for ch in range(2):
                        o_ps[(hp, ch)] = ps_acc.tile(
                            [65, 512], F32, tag="acc", name=f"oacc_p{p}_{hp}_{ch}"
                        )
                for i in range(8):
                    for ch in range(2):
                        pt = {}
                        for hp in range(2):
                            lo, hi = hp * 64, hp * 64 + 64
                            s = ps_mm.tile([128, 512], F32, tag="mm")
                            nc.tensor.matmul(
                                s,
                                qkT[lo:hi, 3 + p, i * 128:(i + 1) * 128],
                                qkT[lo:hi, p, ch * 512:(ch + 1) * 512],
                                start=True,
                                stop=True,
                            )
                            pt[hp] = pt_pool.tile(
                                [128, 512], DT, tag="pt", name=f"pt_{p}_{i}_{ch}_{hp}"
                            )
                            nc.scalar.activation(
                                pt[hp], s, mybir.ActivationFunctionType.Exp,
                                scale=SCALE,
                            )
                        for hp in range(2):
                            h = 2 * p + hp
                            nc.tensor.matmul(
                                o_ps[(hp, ch)],
                                v_sb[:, i, h * 65:h * 65 + 65],
                                pt[hp],
                                start=(i == 0),
                                stop=(i == 7),
                            )
                # normalize: attT rows [hp*64 : hp*64+64] of k-tile p
                for hp in range(2):
                    for ch in range(2):
                        acc = o_ps[(hp, ch)]
                        dinv = stats.tile([65, 512], F32, tag="dinv")
                        nc.vector.reciprocal(dinv[64:65, :], acc[64:65, :])
                        # SBUF sources can't broadcast across partitions;
                        # bounce the denominator row through DRAM.
                        drow = 4 * p + 2 * hp + ch
                        nc.sync.dma_start(
                            out=dinv_dram[drow:drow + 1, :], in_=dinv[64:65, :]
                        )
                        dinv_b = stats.tile([64, 512], F32, tag="dinv_b")
                        nc.sync.dma_start(
                            out=dinv_b,
                            in_=dinv_dram[drow, :].partition_broadcast(64),
                        )
                        if hp == 0:
                            nc.vector.tensor_mul(
                                attT[0:64, p, ch * 512:(ch + 1) * 512],
                                acc[0:64, :],
                                dinv_b,
                            )
                        else:
                            tmp = stats.tile([64, 512], DT, tag="odd_tmp")
                            nc.vector.tensor_mul(tmp, acc[0:64, :], dinv_b)
                            nc.sync.dma_start(
                                out=attT[64:128, p, ch * 512:(ch + 1) * 512],
                                in_=tmp,
                            )

            # ---- phase 4: out = attT^T @ w_o ----
            for i in range(8):
                osb = outsb.tile([128, 768], F32, tag="osb")
                for ch, (c0, cw) in enumerate(((0, 512), (512, 256))):
                    ps = ps_mm.tile([128, 512], F32, tag="mm")
                    for j in range(3):
                        nc.tensor.matmul(
                            ps[:, 0:cw],
                            attT[:, j, i * 128:(i + 1) * 128],
                            wo[:, j, c0:c0 + cw],
                            start=(j == 0),
                            stop=(j == 2),
                        )
                    nc.vector.tensor_copy(osb[:, c0:c0 + cw], ps[:, 0:cw])
                nc.sync.dma_start(out=out_d[i * 128:(i + 1) * 128, :], in_=osb)

    nc.compile()
    return nc


_NC_CACHE = {}


def _get_nc():
    if MODE not in _NC_CACHE:
        _NC_CACHE[MODE] = build_nc(MODE)
    return _NC_CACHE[MODE]


def kernel(x, w_qkv, w_out, b_out):
    x = np.asarray(x, dtype=np.float32)
    w_qkv = np.asarray(w_qkv, dtype=np.float32)
    w_out = np.asarray(w_out, dtype=np.float32)
    b_out = np.asarray(b_out, dtype=np.float32)

    nc = _get_nc()
    ones_col = np.ones((128, 48), dtype=np.float32)
    in_maps = []
    for c in range(NCORES):
        b = c // 2
        hs = (c % 2) * HEADS_PER_CORE
        q_cols = w_qkv[:, hs * DH:(hs + 6) * DH]
        k_cols = w_qkv[:, 768 + hs * DH:768 + (hs + 6) * DH]
        in_maps.append({
            "xT": np.ascontiguousarray(x[b].T),
            "w_qk": np.ascontiguousarray(np.concatenate([q_cols, k_cols], axis=1)),
            "w_v": np.ascontiguousarray(w_qkv[:, 1536 + hs * DH:1536 + (hs + 6) * DH]),
            "w_o": np.ascontiguousarray(w_out[hs * DH:(hs + 6) * DH, :]),
            "ones_col": ones_col,
        })

    res = run_bass_kernel_spmd(nc, in_maps, core_ids=list(range(NCORES))).results

    out = np.empty((4, N, DIM), dtype=np.float32)
    for b in range(4):
        out[b] = res[2 * b]["out"] + res[2 * b + 1]["out"] + b_out
    return out
